# revision 1
# baseline (speedup 1.0000x reference)
"""DeepFM forward kernel for Trainium2, data-parallel over 8 NeuronCores.

Math refactor vs the straightforward DeepFM graph:
  sum_ij fm_interactions[b,i,j] = sum_k (sum_i m[b,i,k]) * (sum_j u[b,j,k])
so the BxNMxNU einsum collapses to an 18-dim per-row dot product of "folded"
tower outputs (16 fold products + the two additive terms via const-1 rows).
The fold is linear, so it is baked into the tower weight matrices host-side:
each tower computes [256 dense | 16 fold | 2 extras] = 274 features per row.

On-chip layout is fully transposed (features on SBUF partitions, batch on the
free dim). The tower outputs then feed the MLP matmuls directly as the moving
operand with no on-chip transposes; the inputs are transposed host-side while
sharding. All matmuls run as float32r (full-rate fp32 for free dim >= 256).

Perf structure: input DMAs ride the scalar-engine HWDGE ring while weights
ride the sync ring (parallel); dummy matmuls on a zeroed tile pre-warm the PE
HAM clock during the initial DMA window; the two towers' narrow extras
matmul groups (M=18) and the two final M=1 matmuls run concurrently in
different PE column strips via tile_position.
"""

import numpy as np

import concourse.bacc as bacc
import concourse.bass as bass  # noqa: F401
import concourse.mybir as mybir
import concourse.tile as tile
from concourse.bass_utils import run_bass_kernel_spmd

N_CORES = 8
B_FULL = 16384
R = B_FULL // N_CORES  # 2048 rows per core
F = 512                # input features per tower
KC = F // 128          # 4 contraction chunks per tower
NT = 512               # batch tile on the free dim
NTILES = R // NT       # 4
DME = 274              # tower output: 256 dense + 16 fold + 2 extras
NX = 18                # fold(16) + [add_m, 1] / [1, add_u] rows
N_WARM = 9             # PE pre-warm matmuls

F32 = mybir.dt.float32
F32R = mybir.dt.float32r

# fp32r weight-pack column offsets ([128, WCOLS] blob)
WM_OFF = 0                    # 4 * 274
WU_OFF = WM_OFF + KC * DME
W1_OFF = WU_OFF + KC * DME    # 4 * 256
W2_OFF = W1_OFF + KC * 256    # 2 * 128
W3_OFF = W2_OFF + 2 * 128     # 1
ONES_OFF = W3_OFF + 1         # 1 (first 18 partitions = 1.0)
WCOLS = ONES_OFF + 1

# fp32 bias-pack column indices ([128, BCOLS])
BM0, BM1, BMX, BU0, BU1, BUX, B1A, B1B, B2C = range(9)
BCOLS = 9


def _tower_ext(W, b, is_movie, b3=0.0):
    """[512,257],[257] -> ([512,274], [274]) with fold + extras columns.

    Extras rows after the 16 fold rows: movie tower emits [additive, const-1],
    user tower emits [const-1, additive]; the FM elementwise product of the
    two 18-row blocks then yields fold products + both additive terms, summed
    by a single ones-vector matmul. The scalar b3 rides on the movie additive
    bias so the final combine needs no separate bias.
    """
    dense_w = W[:, :256]
    fold_w = dense_w.reshape(F, 16, 16).sum(axis=1)        # [512, 16]
    add_w = W[:, 256:257]
    zero_w = np.zeros_like(add_w)
    fold_b = b[:256].reshape(16, 16).sum(axis=0)
    if is_movie:
        tail_w = [add_w, zero_w]
        tail_b = [b[256:257] + b3, np.ones(1, np.float32)]
    else:
        tail_w = [zero_w, add_w]
        tail_b = [np.ones(1, np.float32), b[256:257]]
    w_ext = np.concatenate([dense_w, fold_w, *tail_w], axis=1)
    b_ext = np.concatenate([b[:256], fold_b, *tail_b])
    return w_ext.astype(np.float32), b_ext.astype(np.float32)


def _chunk(Wext):
    """[K, M] -> [128, (K/128)*M]: K-chunk k occupies cols [k*M, (k+1)*M)."""
    kc, m = Wext.shape[0] // 128, Wext.shape[1]
    return Wext.reshape(kc, 128, m).transpose(1, 0, 2).reshape(128, kc * m)


def _col(vec):
    out = np.zeros((128, 1), np.float32)
    out[: len(vec), 0] = vec
    return out


def _pack_weights(Wm, bm, Wu, bu, W1, b1, W2, b2, W3, b3):
    b3v = float(np.asarray(b3, np.float32).reshape(1)[0])
    wm_ext, bm_ext = _tower_ext(Wm, bm, True, b3v)
    wu_ext, bu_ext = _tower_ext(Wu, bu, False)
    ones = np.zeros((128, 1), np.float32)
    ones[:NX, 0] = 1.0
    wp = np.concatenate(
        [
            _chunk(wm_ext),
            _chunk(wu_ext),
            _chunk(W1.astype(np.float32)),
            _chunk(W2.astype(np.float32)),
            W3.astype(np.float32).reshape(128, 1),
            ones,
        ],
        axis=1,
    )
    assert wp.shape == (128, WCOLS), wp.shape
    bp = np.concatenate(
        [
            _col(bm_ext[:128]), _col(bm_ext[128:256]), _col(bm_ext[256:]),
            _col(bu_ext[:128]), _col(bu_ext[128:256]), _col(bu_ext[256:]),
            _col(b1.astype(np.float32)[:128]), _col(b1.astype(np.float32)[128:]),
            _col(b2.astype(np.float32)),
        ],
        axis=1,
    )
    return np.ascontiguousarray(wp), np.ascontiguousarray(bp)


def _build_bass():
    nc = bacc.Bacc()
    xm = nc.dram_tensor("xm", [F, R], F32R, kind="ExternalInput")
    xu = nc.dram_tensor("xu", [F, R], F32R, kind="ExternalInput")
    wp = nc.dram_tensor("wp", [128, WCOLS], F32R, kind="ExternalInput")
    bp = nc.dram_tensor("bp", [128, BCOLS], F32, kind="ExternalInput")
    out = nc.dram_tensor("out", [1, R], F32, kind="ExternalOutput")

    add = mybir.AluOpType.add
    amax = mybir.AluOpType.max
    ident = mybir.ActivationFunctionType.Identity

    with tile.TileContext(nc) as tc:
        with (
            tc.tile_pool(name="wpool", bufs=1) as wpool,
            tc.tile_pool(name="xpool", bufs=3) as xpool,
            tc.tile_pool(name="dpool", bufs=2) as dpool,
            tc.tile_pool(name="opool", bufs=1) as opool,
            tc.tile_pool(name="pspool", bufs=6, space="PSUM") as pspool,
            tc.tile_pool(name="psfin", bufs=1, space="PSUM") as psfin,
            tc.tile_pool(name="pswarm", bufs=1, space="PSUM") as pswarm,
        ):
            # PE pre-warm on a zeroed tile: keeps the HAM clock-gate busy
            # through the initial DMA window so real matmuls start at 2.4 GHz.
            wgar = wpool.tile([128, NT], mybir.dt.bfloat16)
            nc.vector.memset(wgar, 0.0)
            for _ in range(N_WARM):
                pw = pswarm.tile([128, NT], F32, name="psw")
                nc.tensor.matmul(pw, wgar[:, :128], wgar, start=True, stop=True)

            # Weights ride the scalar-engine HWDGE ring in need-order
            # (wm, wu, then the MLP block) while the x tiles get the sync
            # ring to themselves — two HWDGE rings drain in parallel, and
            # neither trigger stream queues behind compute work.
            w = wpool.tile([128, WCOLS], F32R)
            nc.scalar.dma_start(out=w[:, : KC * DME], in_=wp[:, : KC * DME])
            nc.scalar.dma_start(
                out=w[:, KC * DME : 2 * KC * DME], in_=wp[:, KC * DME : 2 * KC * DME]
            )
            b = wpool.tile([128, BCOLS], F32)
            nc.scalar.dma_start(out=b, in_=bp[:, :])
            nc.scalar.dma_start(out=w[:, 2 * KC * DME :], in_=wp[:, 2 * KC * DME :])
            out_sb = opool.tile([1, R], F32)

            xmr = xm.rearrange("(c p) n -> p c n", p=128)
            xur = xu.rearrange("(c p) n -> p c n", p=128)

            for t in range(NTILES):
                n0 = t * NT
                xm_t = xpool.tile([128, KC, NT], F32R, name="xm_t")
                nc.sync.dma_start(out=xm_t, in_=xmr[:, :, n0 : n0 + NT])
                xu_t = xpool.tile([128, KC, NT], F32R, name="xu_t")
                nc.sync.dma_start(out=xu_t, in_=xur[:, :, n0 : n0 + NT])

                # --- tower dense chunks (features x batch, 4 groups) ---
                douts = {}
                for tow, (xt, woff, boff) in enumerate(
                    ((xm_t, WM_OFF, BM0), (xu_t, WU_OFF, BU0))
                ):
                    for c in range(2):
                        c0 = c * 128
                        ps = pspool.tile([128, NT], F32, name="ps_mm")
                        for k in range(KC):
                            lhsT = w[:, woff + k * DME + c0 : woff + k * DME + c0 + 128]
                            nc.tensor.matmul(
                                ps, lhsT, xt[:, k, :],
                                start=(k == 0), stop=(k == KC - 1),
                            )
                        d = dpool.tile([128, NT], F32R, name=f"d{tow}{c}")
                        nc.scalar.activation(
                            out=d, in_=ps, func=ident,
                            bias=b[:, boff + c : boff + c + 1],
                        )
                        douts[(tow, c)] = d

                # --- tower extras: two M=18 groups ---
                psxm = pspool.tile([NX, NT], F32, name="ps_mm")
                psxu = pspool.tile([NX, NT], F32, name="ps_mm")
                for k in range(KC):
                    lm = w[:, WM_OFF + k * DME + 256 : WM_OFF + k * DME + 256 + NX]
                    nc.tensor.matmul(
                        psxm, lm, xm_t[:, k, :], start=(k == 0), stop=(k == KC - 1)
                    )
                for k in range(KC):
                    lu = w[:, WU_OFF + k * DME + 256 : WU_OFF + k * DME + 256 + NX]
                    nc.tensor.matmul(
                        psxu, lu, xu_t[:, k, :], start=(k == 0), stop=(k == KC - 1)
                    )
                dmx = dpool.tile([NX, NT], F32R, name="dmx")
                nc.vector.tensor_scalar_add(out=dmx, in0=psxm, scalar1=b[:NX, BMX : BMX + 1])
                dux = dpool.tile([NX, NT], F32R, name="dux")
                nc.vector.tensor_scalar_add(out=dux, in0=psxu, scalar1=b[:NX, BUX : BUX + 1])
                prod = dpool.tile([NX, NT], F32R, name="prod")
                nc.vector.tensor_mul(out=prod, in0=dmx, in1=dux)

                # --- MLP layer 1: K = [dm0, dm1, du0, du1] ---
                dall = [douts[(0, 0)], douts[(0, 1)], douts[(1, 0)], douts[(1, 1)]]
                h1 = []
                for c in range(2):
                    ps = pspool.tile([128, NT], F32, name="ps_mm")
                    for k in range(4):
                        lhsT = w[:, W1_OFF + k * 256 + c * 128 : W1_OFF + k * 256 + (c + 1) * 128]
                        nc.tensor.matmul(ps, lhsT, dall[k], start=(k == 0), stop=(k == 3))
                    h = dpool.tile([128, NT], F32R, name=f"h1{c}")
                    if c == 0:
                        nc.scalar.activation(
                            out=h, in_=ps,
                            func=mybir.ActivationFunctionType.Relu,
                            bias=b[:, B1A + c : B1A + c + 1],
                        )
                    else:
                        nc.vector.tensor_scalar(
                            out=h, in0=ps, scalar1=b[:, B1A + c : B1A + c + 1],
                            scalar2=0.0, op0=add, op1=amax,
                        )
                    h1.append(h)

                # --- MLP layer 2 ---
                ps = pspool.tile([128, NT], F32, name="ps_mm")
                for k in range(2):
                    lhsT = w[:, W2_OFF + k * 128 : W2_OFF + (k + 1) * 128]
                    nc.tensor.matmul(ps, lhsT, h1[k], start=(k == 0), stop=(k == 1))
                h2 = dpool.tile([128, NT], F32R, name="h2")
                nc.scalar.activation(
                    out=h2, in_=ps,
                    func=mybir.ActivationFunctionType.Relu,
                    bias=b[:, B2C : B2C + 1],
                )

                # --- final: logit = W3.T @ h2 + ones18.T @ prod (b3 rides in
                # the additive bias) ---
                psf = psfin.tile([1, NT], F32, name="ps_fin")
                nc.tensor.matmul(
                    psf, w[:, W3_OFF : W3_OFF + 1], h2, start=True, stop=False
                )
                nc.tensor.matmul(
                    psf, w[:NX, ONES_OFF : ONES_OFF + 1], prod, start=False, stop=True
                )
                nc.vector.tensor_copy(out_sb[:, n0 : n0 + NT], psf)
                nc.scalar.dma_start(
                    out=out[:, n0 : n0 + NT], in_=out_sb[:, n0 : n0 + NT]
                )
    nc.finalize()
    return nc


_NC_CACHE = []


def kernel(movie_vectors, user_vectors, Wm, bm, Wu, bu, W1, b1, W2, b2, W3, b3):
    movie_vectors = np.asarray(movie_vectors, np.float32)
    user_vectors = np.asarray(user_vectors, np.float32)
    wp, bp = _pack_weights(
        np.asarray(Wm, np.float32), np.asarray(bm, np.float32),
        np.asarray(Wu, np.float32), np.asarray(bu, np.float32),
        np.asarray(W1, np.float32), np.asarray(b1, np.float32),
        np.asarray(W2, np.float32), np.asarray(b2, np.float32),
        np.asarray(W3, np.float32), np.asarray(b3, np.float32),
    )
    xmT = np.ascontiguousarray(movie_vectors.T)  # [512, 16384]
    xuT = np.ascontiguousarray(user_vectors.T)

    if not _NC_CACHE:
        _NC_CACHE.append(_build_bass())
    nc = _NC_CACHE[0]

    in_maps = []
    for c in range(N_CORES):
        sl = slice(c * R, (c + 1) * R)
        in_maps.append(
            {
                "xm": np.ascontiguousarray(xmT[:, sl]),
                "xu": np.ascontiguousarray(xuT[:, sl]),
                "wp": wp,
                "bp": bp,
            }
        )
    res = run_bass_kernel_spmd(nc, in_maps, core_ids=list(range(N_CORES)))
    kernel.last_result = res
    return np.concatenate([r["out"].reshape(R, 1) for r in res.results], axis=0)



# revision 3
# speedup vs baseline: 1.4360x; 1.4360x over previous
"""DeepFM forward kernel for Trainium2, data-parallel over 8 NeuronCores.

Math refactor vs the straightforward DeepFM graph:
  sum_ij fm_interactions[b,i,j] = sum_k (sum_i m[b,i,k]) * (sum_j u[b,j,k])
so the BxNMxNU einsum collapses to an 18-dim per-row dot product of "folded"
tower outputs (16 fold products + the two additive terms via const-1 rows).

Additionally, because the first MLP nonlinearity comes after W1, the tower
dense outputs are never materialized: W1 is folded into the tower weights
host-side (Wm1 = Wm_dense @ W1_top, Wu1 = Wu_dense @ W1_bot), so the PE
computes h1_pre directly from the inputs. This cuts the per-tile matmul
passes from 36 to 28.

All on-chip operands are bf16 (fp32 PSUM accumulation): halves HBM traffic
and lets FWL hide the LDWEIGHTS behind the previous matmul's stream.
Inputs are transposed + chunk-packed host-side so every input DMA is
contiguous 4KB-per-partition lines; weight DMAs are split and ordered by
first use so the first matmul can start as soon as possible.
"""

import numpy as np
import ml_dtypes

import concourse.bacc as bacc
import concourse.bass as bass  # noqa: F401
import concourse.mybir as mybir
import concourse.tile as tile
from concourse.bass_utils import run_bass_kernel_spmd

N_CORES = 8
B_FULL = 16384
R = B_FULL // N_CORES  # 2048 rows per core
F = 512                # input features per tower
KC = F // 128          # 4 contraction chunks per tower
NT = 512               # batch tile on the free dim
NTILES = R // NT       # 4
NX = 18                # fold(16) + [add_m, 1] / [1, add_u] rows
N_WARM = 3             # PE pre-warm matmuls

F32 = mybir.dt.float32
BF16 = mybir.dt.bfloat16
BF16_NP = ml_dtypes.bfloat16

# bf16 weight-pack column offsets ([128, WCOLS] blob)
# h1a: 8 chunks (4 xm + 4 xu) x 128 cols; h1b likewise; then the two
# 18-col extras blocks, W2 (2 x 128), W3 (1), ones18 (1).
H1A_OFF = 0
H1B_OFF = H1A_OFF + 8 * 128     # 1024
EXM_OFF = H1B_OFF + 8 * 128     # 2048
EXU_OFF = EXM_OFF + KC * NX     # 2120
W2_OFF = EXU_OFF + KC * NX      # 2192
W3_OFF = W2_OFF + 2 * 128       # 2448
ONES_OFF = W3_OFF + 1           # 2449
WCOLS = ONES_OFF + 1            # 2450

# fp32 bias-pack column indices ([128, BCOLS])
B1A, B1B, B2C, BMX, BUX = range(5)
BCOLS = 5


def _fold_ext(W, b, is_movie, b3=0.0):
    """[512,257],[257] -> ([512,18], [18]) extras weights/bias.

    Rows after the 16 fold rows: movie emits [additive(+b3 bias), const-1],
    user emits [const-1, additive]; the FM elementwise product of the two
    18-row blocks then yields fold products + both additive terms + b3,
    summed by a single ones-vector matmul.
    """
    dense_w = W[:, :256]
    fold_w = dense_w.reshape(F, 16, 16).sum(axis=1)        # [512, 16]
    add_w = W[:, 256:257]
    zero_w = np.zeros_like(add_w)
    fold_b = b[:256].reshape(16, 16).sum(axis=0)
    if is_movie:
        tail_w = [add_w, zero_w]
        tail_b = [b[256:257] + b3, np.ones(1, np.float32)]
    else:
        tail_w = [zero_w, add_w]
        tail_b = [np.ones(1, np.float32), b[256:257]]
    w_ext = np.concatenate([fold_w, *tail_w], axis=1)
    b_ext = np.concatenate([fold_b, *tail_b])
    return w_ext.astype(np.float32), b_ext.astype(np.float32)


def _chunk(Wext):
    """[K, M] -> [128, (K/128)*M]: K-chunk k occupies cols [k*M, (k+1)*M)."""
    kc, m = Wext.shape[0] // 128, Wext.shape[1]
    return Wext.reshape(kc, 128, m).transpose(1, 0, 2).reshape(128, kc * m)


def _col(vec):
    out = np.zeros((128, 1), np.float32)
    out[: len(vec), 0] = vec
    return out


def _pack_weights(Wm, bm, Wu, bu, W1, b1, W2, b2, W3, b3):
    b3v = float(np.asarray(b3, np.float32).reshape(1)[0])
    W1t, W1b = W1[:256], W1[256:]
    Wm1 = Wm[:, :256] @ W1t                        # [512, 256] fused tower+W1
    Wu1 = Wu[:, :256] @ W1b
    b1p = b1 + bm[:256] @ W1t + bu[:256] @ W1b     # [256]
    exm_w, exm_b = _fold_ext(Wm, bm, True, b3v)
    exu_w, exu_b = _fold_ext(Wu, bu, False)
    ones = np.zeros((128, 1), np.float32)
    ones[:NX, 0] = 1.0

    def h1block(col0):
        # 8 chunks x 128 cols: xm chunks of Wm1 then xu chunks of Wu1
        return np.concatenate(
            [_chunk(Wm1[:, col0 : col0 + 128]), _chunk(Wu1[:, col0 : col0 + 128])],
            axis=1,
        )

    wp = np.concatenate(
        [
            h1block(0),
            h1block(128),
            _chunk(exm_w),
            _chunk(exu_w),
            _chunk(W2.astype(np.float32)),
            W3.astype(np.float32).reshape(128, 1),
            ones,
        ],
        axis=1,
    )
    assert wp.shape == (128, WCOLS), wp.shape
    bp = np.concatenate(
        [
            _col(b1p[:128]), _col(b1p[128:]), _col(b2.astype(np.float32)),
            _col(exm_b), _col(exu_b),
        ],
        axis=1,
    )
    return wp.astype(BF16_NP), np.ascontiguousarray(bp.astype(np.float32))


def _build_bass():
    nc = bacc.Bacc()
    xm = nc.dram_tensor("xm", [128, NTILES * KC * NT], BF16, kind="ExternalInput")
    xu = nc.dram_tensor("xu", [128, NTILES * KC * NT], BF16, kind="ExternalInput")
    wp = nc.dram_tensor("wp", [128, WCOLS], BF16, kind="ExternalInput")
    bp = nc.dram_tensor("bp", [128, BCOLS], F32, kind="ExternalInput")
    out = nc.dram_tensor("out", [1, R], F32, kind="ExternalOutput")

    relu = mybir.ActivationFunctionType.Relu
    CT = KC * NT  # 2048 free-dim cols per x tile

    with tile.TileContext(nc) as tc:
        with (
            tc.tile_pool(name="wpool", bufs=1) as wpool,
            tc.tile_pool(name="xpool", bufs=1) as xpool,
            tc.tile_pool(name="dpool", bufs=2) as dpool,
            tc.tile_pool(name="epool", bufs=2) as epool,
            tc.tile_pool(name="opool", bufs=1) as opool,
            tc.tile_pool(name="ps1", bufs=3, space="PSUM") as ps1,
            tc.tile_pool(name="psx", bufs=2, space="PSUM") as psx,
            tc.tile_pool(name="ps2", bufs=1, space="PSUM") as ps2p,
            tc.tile_pool(name="psf", bufs=1, space="PSUM") as psfp,
            tc.tile_pool(name="pswarm", bufs=1, space="PSUM") as pswarm,
        ):
            # PE pre-warm on a zeroed tile: keeps the PE busy through the
            # initial DMA window so the HAM un-throttles earlier.
            wgar = wpool.tile([128, NT], BF16)
            nc.vector.memset(wgar, 0.0)
            for _ in range(N_WARM):
                pw = pswarm.tile([128, NT], F32, name="psw")
                nc.tensor.matmul(pw, wgar[:, :128], wgar, start=True, stop=True)

            w = wpool.tile([128, WCOLS], BF16)
            b = wpool.tile([128, BCOLS], F32)
            # Weight/bias DMAs on the scalar HWDGE ring, ordered by first
            # use; x tiles for the user tower share this ring, movie tower
            # rides the sync ring.
            nc.scalar.dma_start(out=w[:, :512], in_=wp[:, :512])

            xm_t = [xpool.tile([128, CT], BF16, name=f"xm{t}") for t in range(NTILES)]
            xu_t = [xpool.tile([128, CT], BF16, name=f"xu{t}") for t in range(NTILES)]
            xmr = xm.rearrange("p (t c) -> p t c", t=NTILES)
            xur = xu.rearrange("p (t c) -> p t c", t=NTILES)

            # tile 0 split in halves for an earlier first matmul
            nc.sync.dma_start(out=xm_t[0][:, : CT // 2], in_=xmr[:, 0, : CT // 2])
            nc.sync.dma_start(out=xm_t[0][:, CT // 2 :], in_=xmr[:, 0, CT // 2 :])
            nc.scalar.dma_start(out=xu_t[0][:, : CT // 2], in_=xur[:, 0, : CT // 2])
            nc.scalar.dma_start(out=w[:, 512:2048], in_=wp[:, 512:2048])
            nc.scalar.dma_start(out=xu_t[0][:, CT // 2 :], in_=xur[:, 0, CT // 2 :])
            nc.scalar.dma_start(out=b, in_=bp[:, :])
            nc.scalar.dma_start(out=w[:, 2048:], in_=wp[:, 2048:])
            for t in range(1, NTILES):
                nc.sync.dma_start(out=xm_t[t], in_=xmr[:, t, :])
                nc.scalar.dma_start(out=xu_t[t], in_=xur[:, t, :])

            out_sb = opool.tile([1, R], F32)

            for t in range(NTILES):
                n0 = t * NT
                xmt, xut = xm_t[t], xu_t[t]

                # --- fused tower+W1: two [128, NT] groups, 8 passes each ---
                ps1a = ps1.tile([128, NT], F32, name="ps_mm")
                ps1b = ps1.tile([128, NT], F32, name="ps_mm")
                for ps, off in ((ps1a, H1A_OFF), (ps1b, H1B_OFF)):
                    for k in range(KC):
                        nc.tensor.matmul(
                            ps, w[:, off + k * 128 : off + (k + 1) * 128],
                            xmt[:, k * NT : (k + 1) * NT],
                            start=(k == 0), stop=False,
                        )
                    for k in range(KC):
                        nc.tensor.matmul(
                            ps, w[:, off + (4 + k) * 128 : off + (5 + k) * 128],
                            xut[:, k * NT : (k + 1) * NT],
                            start=False, stop=(k == KC - 1),
                        )
                h1a = dpool.tile([128, NT], BF16, name="h1a")
                nc.scalar.activation(out=h1a, in_=ps1a, func=relu, bias=b[:, B1A : B1A + 1])
                h1b = dpool.tile([128, NT], BF16, name="h1b")
                nc.scalar.activation(out=h1b, in_=ps1b, func=relu, bias=b[:, B1B : B1B + 1])

                # --- tower extras: two M=18 groups ---
                psxm = psx.tile([NX, NT], F32, name="ps_x")
                psxu = psx.tile([NX, NT], F32, name="ps_x")
                for k in range(KC):
                    lm = w[:, EXM_OFF + k * NX : EXM_OFF + (k + 1) * NX]
                    nc.tensor.matmul(
                        psxm, lm, xmt[:, k * NT : (k + 1) * NT],
                        start=(k == 0), stop=(k == KC - 1),
                    )
                for k in range(KC):
                    lu = w[:, EXU_OFF + k * NX : EXU_OFF + (k + 1) * NX]
                    nc.tensor.matmul(
                        psxu, lu, xut[:, k * NT : (k + 1) * NT],
                        start=(k == 0), stop=(k == KC - 1),
                    )
                dmx = epool.tile([NX, NT], BF16, name="dmx")
                nc.vector.tensor_scalar_add(out=dmx, in0=psxm, scalar1=b[:NX, BMX : BMX + 1])
                dux = epool.tile([NX, NT], BF16, name="dux")
                nc.vector.tensor_scalar_add(out=dux, in0=psxu, scalar1=b[:NX, BUX : BUX + 1])
                prod = epool.tile([NX, NT], BF16, name="prod")
                nc.vector.tensor_mul(out=prod, in0=dmx, in1=dux)

                # --- MLP layer 2 ---
                ps2 = ps2p.tile([128, NT], F32, name="ps_l2")
                nc.tensor.matmul(ps2, w[:, W2_OFF : W2_OFF + 128], h1a, start=True, stop=False)
                nc.tensor.matmul(ps2, w[:, W2_OFF + 128 : W2_OFF + 256], h1b, start=False, stop=True)
                h2 = dpool.tile([128, NT], BF16, name="h2")
                nc.scalar.activation(out=h2, in_=ps2, func=relu, bias=b[:, B2C : B2C + 1])

                # --- final: logit = W3.T @ h2 + ones18.T @ prod ---
                psf = psfp.tile([1, NT], F32, name="ps_fin")
                nc.tensor.matmul(psf, w[:, W3_OFF : W3_OFF + 1], h2, start=True, stop=False)
                nc.tensor.matmul(psf, w[:NX, ONES_OFF : ONES_OFF + 1], prod, start=False, stop=True)
                nc.vector.tensor_copy(out_sb[:, n0 : n0 + NT], psf)
                nc.sync.dma_start(out=out[:, n0 : n0 + NT], in_=out_sb[:, n0 : n0 + NT])
    nc.finalize()
    return nc


def _pack_x(xT):
    """[512, 2048] fp32 -> [128, NTILES*KC*NT] bf16, tile/chunk packed so
    each tile's DMA reads 4KB contiguous per partition."""
    a = xT.reshape(KC, 128, NTILES, NT).transpose(1, 2, 0, 3)
    return np.ascontiguousarray(a.reshape(128, NTILES * KC * NT).astype(BF16_NP))


_NC_CACHE = []


def kernel(movie_vectors, user_vectors, Wm, bm, Wu, bu, W1, b1, W2, b2, W3, b3):
    movie_vectors = np.asarray(movie_vectors, np.float32)
    user_vectors = np.asarray(user_vectors, np.float32)
    wp, bp = _pack_weights(
        np.asarray(Wm, np.float32), np.asarray(bm, np.float32),
        np.asarray(Wu, np.float32), np.asarray(bu, np.float32),
        np.asarray(W1, np.float32), np.asarray(b1, np.float32),
        np.asarray(W2, np.float32), np.asarray(b2, np.float32),
        np.asarray(W3, np.float32), np.asarray(b3, np.float32),
    )
    xmT = np.ascontiguousarray(movie_vectors.T)  # [512, 16384]
    xuT = np.ascontiguousarray(user_vectors.T)

    if not _NC_CACHE:
        _NC_CACHE.append(_build_bass())
    nc = _NC_CACHE[0]

    in_maps = []
    for c in range(N_CORES):
        sl = slice(c * R, (c + 1) * R)
        in_maps.append(
            {
                "xm": _pack_x(xmT[:, sl]),
                "xu": _pack_x(xuT[:, sl]),
                "wp": wp,
                "bp": bp,
            }
        )
    res = run_bass_kernel_spmd(nc, in_maps, core_ids=list(range(N_CORES)))
    kernel.last_result = res
    return np.concatenate([r["out"].reshape(R, 1) for r in res.results], axis=0)


# revision 4
# speedup vs baseline: 1.4613x; 1.0176x over previous
"""DeepFM forward kernel for Trainium2, data-parallel over 8 NeuronCores.

Math refactor vs the straightforward DeepFM graph:
  sum_ij fm_interactions[b,i,j] = sum_k (sum_i m[b,i,k]) * (sum_j u[b,j,k])
so the BxNMxNU einsum collapses to an 18-dim per-row dot product of "folded"
tower outputs (16 fold products + the two additive terms via const-1 rows).

Additionally, because the first MLP nonlinearity comes after W1, the tower
dense outputs are never materialized: W1 is folded into the tower weights
host-side (Wm1 = Wm_dense @ W1_top, Wu1 = Wu_dense @ W1_bot), so the PE
computes h1_pre directly from the inputs. This cuts the per-tile matmul
passes from 36 to 28.

All on-chip operands are bf16 (fp32 PSUM accumulation): halves HBM traffic
and lets FWL hide the LDWEIGHTS behind the previous matmul's stream.
Inputs are transposed + chunk-packed host-side so every input DMA is
contiguous 4KB-per-partition lines; weight DMAs are split and ordered by
first use so the first matmul can start as soon as possible.
"""

import numpy as np
import ml_dtypes

import concourse.bacc as bacc
import concourse.bass as bass  # noqa: F401
import concourse.mybir as mybir
import concourse.tile as tile
from concourse.bass_utils import run_bass_kernel_spmd

N_CORES = 8
B_FULL = 16384
R = B_FULL // N_CORES  # 2048 rows per core
F = 512                # input features per tower
KC = F // 128          # 4 contraction chunks per tower
NT = 512               # batch tile on the free dim
NTILES = R // NT       # 4
NX = 18                # fold(16) + [add_m, 1] / [1, add_u] rows
N_WARM = 3             # PE pre-warm matmuls

F32 = mybir.dt.float32
BF16 = mybir.dt.bfloat16
BF16_NP = ml_dtypes.bfloat16

# bf16 weight-pack column offsets ([128, WCOLS] blob)
# h1a: 8 chunks (4 xm + 4 xu) x 128 cols; h1b likewise; then the two
# 18-col extras blocks, W2 (2 x 128), W3 (1), ones18 (1).
H1A_OFF = 0
H1B_OFF = H1A_OFF + 8 * 128     # 1024
EXM_OFF = H1B_OFF + 8 * 128     # 2048
EXU_OFF = EXM_OFF + KC * NX     # 2120
W2_OFF = EXU_OFF + KC * NX      # 2192
W3_OFF = W2_OFF + 2 * 128       # 2448
ONES_OFF = W3_OFF + 1           # 2449
WCOLS = ONES_OFF + 1            # 2450

# fp32 bias-pack column indices ([128, BCOLS])
B1A, B1B, B2C, BMX, BUX = range(5)
BCOLS = 5


def _fold_ext(W, b, is_movie, b3=0.0):
    """[512,257],[257] -> ([512,18], [18]) extras weights/bias.

    Rows after the 16 fold rows: movie emits [additive(+b3 bias), const-1],
    user emits [const-1, additive]; the FM elementwise product of the two
    18-row blocks then yields fold products + both additive terms + b3,
    summed by a single ones-vector matmul.
    """
    dense_w = W[:, :256]
    fold_w = dense_w.reshape(F, 16, 16).sum(axis=1)        # [512, 16]
    add_w = W[:, 256:257]
    zero_w = np.zeros_like(add_w)
    fold_b = b[:256].reshape(16, 16).sum(axis=0)
    if is_movie:
        tail_w = [add_w, zero_w]
        tail_b = [b[256:257] + b3, np.ones(1, np.float32)]
    else:
        tail_w = [zero_w, add_w]
        tail_b = [np.ones(1, np.float32), b[256:257]]
    w_ext = np.concatenate([fold_w, *tail_w], axis=1)
    b_ext = np.concatenate([fold_b, *tail_b])
    return w_ext.astype(np.float32), b_ext.astype(np.float32)


def _chunk(Wext):
    """[K, M] -> [128, (K/128)*M]: K-chunk k occupies cols [k*M, (k+1)*M)."""
    kc, m = Wext.shape[0] // 128, Wext.shape[1]
    return Wext.reshape(kc, 128, m).transpose(1, 0, 2).reshape(128, kc * m)


def _col(vec):
    out = np.zeros((128, 1), np.float32)
    out[: len(vec), 0] = vec
    return out


def _pack_weights(Wm, bm, Wu, bu, W1, b1, W2, b2, W3, b3):
    b3v = float(np.asarray(b3, np.float32).reshape(1)[0])
    W1t, W1b = W1[:256], W1[256:]
    Wm1 = Wm[:, :256] @ W1t                        # [512, 256] fused tower+W1
    Wu1 = Wu[:, :256] @ W1b
    b1p = b1 + bm[:256] @ W1t + bu[:256] @ W1b     # [256]
    exm_w, exm_b = _fold_ext(Wm, bm, True, b3v)
    exu_w, exu_b = _fold_ext(Wu, bu, False)
    ones = np.zeros((128, 1), np.float32)
    ones[:NX, 0] = 1.0

    def h1block(col0):
        # 8 chunks x 128 cols: xm chunks of Wm1 then xu chunks of Wu1
        return np.concatenate(
            [_chunk(Wm1[:, col0 : col0 + 128]), _chunk(Wu1[:, col0 : col0 + 128])],
            axis=1,
        )

    wp = np.concatenate(
        [
            h1block(0),
            h1block(128),
            _chunk(exm_w),
            _chunk(exu_w),
            _chunk(W2.astype(np.float32)),
            W3.astype(np.float32).reshape(128, 1),
            ones,
        ],
        axis=1,
    )
    assert wp.shape == (128, WCOLS), wp.shape
    bp = np.concatenate(
        [
            _col(b1p[:128]), _col(b1p[128:]), _col(b2.astype(np.float32)),
            _col(exm_b), _col(exu_b),
        ],
        axis=1,
    )
    return wp.astype(BF16_NP), np.ascontiguousarray(bp.astype(np.float32))


def _build_bass():
    nc = bacc.Bacc()
    xm = nc.dram_tensor("xm", [128, NTILES * KC * NT], BF16, kind="ExternalInput")
    xu = nc.dram_tensor("xu", [128, NTILES * KC * NT], BF16, kind="ExternalInput")
    wp = nc.dram_tensor("wp", [128, WCOLS], BF16, kind="ExternalInput")
    bp = nc.dram_tensor("bp", [128, BCOLS], F32, kind="ExternalInput")
    out = nc.dram_tensor("out", [1, R], F32, kind="ExternalOutput")

    relu = mybir.ActivationFunctionType.Relu
    CT = KC * NT  # 2048 free-dim cols per x tile

    with tile.TileContext(nc) as tc:
        with (
            tc.tile_pool(name="wpool", bufs=1) as wpool,
            tc.tile_pool(name="xpool", bufs=1) as xpool,
            tc.tile_pool(name="dpool", bufs=2) as dpool,
            tc.tile_pool(name="epool", bufs=2) as epool,
            tc.tile_pool(name="opool", bufs=1) as opool,
            tc.tile_pool(name="ps1", bufs=3, space="PSUM") as ps1,
            tc.tile_pool(name="psx", bufs=2, space="PSUM") as psx,
            tc.tile_pool(name="ps2", bufs=1, space="PSUM") as ps2p,
            tc.tile_pool(name="psf", bufs=1, space="PSUM") as psfp,
            tc.tile_pool(name="pswarm", bufs=1, space="PSUM") as pswarm,
        ):
            # PE pre-warm on a zeroed tile: keeps the PE busy through the
            # initial DMA window so the HAM un-throttles earlier.
            wgar = wpool.tile([128, NT], BF16)
            nc.vector.memset(wgar, 0.0)
            for _ in range(N_WARM):
                pw = pswarm.tile([128, NT], F32, name="psw")
                nc.tensor.matmul(pw, wgar[:, :128], wgar, start=True, stop=True)

            w = wpool.tile([128, WCOLS], BF16)
            b = wpool.tile([128, BCOLS], F32)
            # Weight/bias DMAs on the scalar HWDGE ring, ordered by first
            # use; x tiles for the user tower share this ring, movie tower
            # rides the sync ring.
            # All weights/bias first on the scalar ring (they gate the PE
            # stream); xu tiles follow. xm rides the sync ring in parallel.
            nc.scalar.dma_start(out=w[:, :1024], in_=wp[:, :1024])
            nc.scalar.dma_start(out=w[:, 1024:2048], in_=wp[:, 1024:2048])
            nc.scalar.dma_start(out=b, in_=bp[:, :])
            nc.scalar.dma_start(out=w[:, 2048:], in_=wp[:, 2048:])

            xm_t = [xpool.tile([128, CT], BF16, name=f"xm{t}") for t in range(NTILES)]
            xu_t = [xpool.tile([128, CT], BF16, name=f"xu{t}") for t in range(NTILES)]
            xmr = xm.rearrange("p (t c) -> p t c", t=NTILES)
            xur = xu.rearrange("p (t c) -> p t c", t=NTILES)

            # tile 0 split in halves for an earlier first matmul
            nc.sync.dma_start(out=xm_t[0][:, : CT // 2], in_=xmr[:, 0, : CT // 2])
            nc.sync.dma_start(out=xm_t[0][:, CT // 2 :], in_=xmr[:, 0, CT // 2 :])
            nc.scalar.dma_start(out=xu_t[0][:, : CT // 2], in_=xur[:, 0, : CT // 2])
            nc.scalar.dma_start(out=xu_t[0][:, CT // 2 :], in_=xur[:, 0, CT // 2 :])
            for t in range(1, NTILES):
                nc.sync.dma_start(out=xm_t[t], in_=xmr[:, t, :])
                nc.scalar.dma_start(out=xu_t[t], in_=xur[:, t, :])

            out_sb = opool.tile([1, R], F32)

            for t in range(NTILES):
                n0 = t * NT
                xmt, xut = xm_t[t], xu_t[t]

                # --- fused tower+W1: two [128, NT] groups, 8 passes each ---
                ps1a = ps1.tile([128, NT], F32, name="ps_mm")
                ps1b = ps1.tile([128, NT], F32, name="ps_mm")
                for ps, off in ((ps1a, H1A_OFF), (ps1b, H1B_OFF)):
                    for k in range(KC):
                        nc.tensor.matmul(
                            ps, w[:, off + k * 128 : off + (k + 1) * 128],
                            xmt[:, k * NT : (k + 1) * NT],
                            start=(k == 0), stop=False,
                        )
                    for k in range(KC):
                        nc.tensor.matmul(
                            ps, w[:, off + (4 + k) * 128 : off + (5 + k) * 128],
                            xut[:, k * NT : (k + 1) * NT],
                            start=False, stop=(k == KC - 1),
                        )
                h1a = dpool.tile([128, NT], BF16, name="h1a")
                nc.scalar.activation(out=h1a, in_=ps1a, func=relu, bias=b[:, B1A : B1A + 1])
                h1b = dpool.tile([128, NT], BF16, name="h1b")
                nc.scalar.activation(out=h1b, in_=ps1b, func=relu, bias=b[:, B1B : B1B + 1])

                # --- tower extras: two M=18 groups ---
                psxm = psx.tile([NX, NT], F32, name="ps_x")
                psxu = psx.tile([NX, NT], F32, name="ps_x")
                for k in range(KC):
                    lm = w[:, EXM_OFF + k * NX : EXM_OFF + (k + 1) * NX]
                    nc.tensor.matmul(
                        psxm, lm, xmt[:, k * NT : (k + 1) * NT],
                        start=(k == 0), stop=(k == KC - 1),
                    )
                for k in range(KC):
                    lu = w[:, EXU_OFF + k * NX : EXU_OFF + (k + 1) * NX]
                    nc.tensor.matmul(
                        psxu, lu, xut[:, k * NT : (k + 1) * NT],
                        start=(k == 0), stop=(k == KC - 1),
                    )
                dmx = epool.tile([NX, NT], BF16, name="dmx")
                nc.vector.tensor_scalar_add(out=dmx, in0=psxm, scalar1=b[:NX, BMX : BMX + 1])
                dux = epool.tile([NX, NT], BF16, name="dux")
                nc.vector.tensor_scalar_add(out=dux, in0=psxu, scalar1=b[:NX, BUX : BUX + 1])
                prod = epool.tile([NX, NT], BF16, name="prod")
                nc.vector.tensor_mul(out=prod, in0=dmx, in1=dux)

                # --- MLP layer 2 ---
                ps2 = ps2p.tile([128, NT], F32, name="ps_l2")
                nc.tensor.matmul(ps2, w[:, W2_OFF : W2_OFF + 128], h1a, start=True, stop=False)
                nc.tensor.matmul(ps2, w[:, W2_OFF + 128 : W2_OFF + 256], h1b, start=False, stop=True)
                h2 = dpool.tile([128, NT], BF16, name="h2")
                nc.scalar.activation(out=h2, in_=ps2, func=relu, bias=b[:, B2C : B2C + 1])

                # --- final: logit = W3.T @ h2 + ones18.T @ prod ---
                psf = psfp.tile([1, NT], F32, name="ps_fin")
                nc.tensor.matmul(psf, w[:, W3_OFF : W3_OFF + 1], h2, start=True, stop=False)
                nc.tensor.matmul(psf, w[:NX, ONES_OFF : ONES_OFF + 1], prod, start=False, stop=True)
                nc.vector.tensor_copy(out_sb[:, n0 : n0 + NT], psf)
                nc.sync.dma_start(out=out[:, n0 : n0 + NT], in_=out_sb[:, n0 : n0 + NT])
    nc.finalize()
    return nc


def _pack_x(xT):
    """[512, 2048] fp32 -> [128, NTILES*KC*NT] bf16, tile/chunk packed so
    each tile's DMA reads 4KB contiguous per partition."""
    a = xT.reshape(KC, 128, NTILES, NT).transpose(1, 2, 0, 3)
    return np.ascontiguousarray(a.reshape(128, NTILES * KC * NT).astype(BF16_NP))


_NC_CACHE = []


def kernel(movie_vectors, user_vectors, Wm, bm, Wu, bu, W1, b1, W2, b2, W3, b3):
    movie_vectors = np.asarray(movie_vectors, np.float32)
    user_vectors = np.asarray(user_vectors, np.float32)
    wp, bp = _pack_weights(
        np.asarray(Wm, np.float32), np.asarray(bm, np.float32),
        np.asarray(Wu, np.float32), np.asarray(bu, np.float32),
        np.asarray(W1, np.float32), np.asarray(b1, np.float32),
        np.asarray(W2, np.float32), np.asarray(b2, np.float32),
        np.asarray(W3, np.float32), np.asarray(b3, np.float32),
    )
    xmT = np.ascontiguousarray(movie_vectors.T)  # [512, 16384]
    xuT = np.ascontiguousarray(user_vectors.T)

    if not _NC_CACHE:
        _NC_CACHE.append(_build_bass())
    nc = _NC_CACHE[0]

    in_maps = []
    for c in range(N_CORES):
        sl = slice(c * R, (c + 1) * R)
        in_maps.append(
            {
                "xm": _pack_x(xmT[:, sl]),
                "xu": _pack_x(xuT[:, sl]),
                "wp": wp,
                "bp": bp,
            }
        )
    res = run_bass_kernel_spmd(nc, in_maps, core_ids=list(range(N_CORES)))
    kernel.last_result = res
    return np.concatenate([r["out"].reshape(R, 1) for r in res.results], axis=0)
